# revision 5
# baseline (speedup 1.0000x reference)
"""Causal single-head self-attention on 8 Trainium2 NeuronCores.

Problem: x:[8,2048,1024], Wq/Wk/Wv:[1024,64] ->
    out[b] = softmax(tril(x[b]Wq (x[b]Wk)^T / 64)) @ (x[b]Wv)   [8,2048,64]

Sharding: data-parallel over batch -- core b gets batch element b.
Weights replicated.

Per-core algorithm (all layouts chosen so no on-device transposes of x or
of the attention matrix are ever needed):
  - host pre-transposes x[b] -> xT [E, S] (pure layout change)
  - qkT[128, S]: rows 0:64 = q^T, 64:128 = k^T via one packed projection
    matmul per (e-chunk, s-block): psum += [Wq|Wk][e].T @ xT[e]
  - kT copied (SBUF->SBUF DMA) to partitions 0:64 so it can be a matmul
    lhsT at base partition 0
  - v^T projected the same way, then PE-transposed into natural layout
    v[s,64], augmented with a ones column -> v_aug[s, 65]
  - scores^T[k-block, q-block] = kT_block.T @ qT_block  (K=64)
    exp via ACT (scale=1/64 folded in); causal mask on the 4 diagonal
    blocks per q-block via gpsimd affine_select (fill 0 after exp)
  - out^T psum[65, qb] += v_aug[kc].T @ expT  accumulated over k-chunks;
    row 64 = softmax denominators (ones column trick)
  - PE-transpose out^T -> [128, 65]; divide rows 0:64 by col 64
    (per-partition reciprocal+scale on DVE); DMA natural [S, 64] out.
"""

import os
from contextlib import ExitStack

import numpy as np

import concourse.bass as bass
import concourse.mybir as mybir
import concourse.tile as tile
from concourse import bacc
from concourse.bass_utils import run_bass_kernel_spmd
from concourse.masks import make_identity

B, S, E, H = 8, 2048, 1024, 64
P = 128
QB = 512  # q-block (psum free dim)
F32 = mybir.dt.float32


def build_kernel_body(tc, xT_d, wqk_d, wv_d, out_d, s=S, e_dim=E):
    nc = tc.nc
    EC = e_dim // P  # e-chunks
    NQB = s // QB    # q-blocks
    NT = s // P      # s-tiles of 128

    ctx = ExitStack()
    with ctx:
        const = ctx.enter_context(tc.tile_pool(name="const", bufs=1))
        big = ctx.enter_context(tc.tile_pool(name="big", bufs=1))

        wqk_sb = const.tile([P, EC, 2 * H], F32)
        nc.sync.dma_start(wqk_sb[:], wqk_d.rearrange("(c p) m -> p c m", p=P))
        wv_sb = const.tile([P, EC, H], F32)
        nc.sync.dma_start(wv_sb[:], wv_d.rearrange("(c p) m -> p c m", p=P))
        ident = const.tile([P, P], F32)
        make_identity(nc, ident[:])

        xT_sb = big.tile([P, EC, s], F32)
        for ec in range(EC):
            nc.sync.dma_start(xT_sb[:, ec, :], xT_d[ec * P:(ec + 1) * P, :])

        qkT_sb = big.tile([P, s], F32)   # rows 0:64 qT, rows 64:128 kT
        kT_sb = big.tile([H, s], F32)    # kT at base partition 0
        vT_sb = big.tile([H, s], F32)
        v_sb = big.tile([P, NT, H + 1], F32)  # natural v + ones col
        out_sb = big.tile([P, NT, H], F32)

        # ---------------- Phase 1: projections ----------------
        with tc.tile_pool(name="ps_qk", bufs=1, space="PSUM") as pqk, \
             tc.tile_pool(name="ps_vt", bufs=1, space="PSUM") as pvt:
            psum_qk = [pqk.tile([P, QB], F32, name=f"psum_qk{i}") for i in range(NQB)]
            psum_vT = [pvt.tile([H, QB], F32, name=f"psum_vT{i}") for i in range(NQB)]
            for ec in range(EC):
                for qb in range(NQB):
                    nc.tensor.matmul(
                        psum_qk[qb][:],
                        lhsT=wqk_sb[:, ec, :],
                        rhs=xT_sb[:, ec, qb * QB:(qb + 1) * QB],
                        start=(ec == 0), stop=(ec == EC - 1),
                    )
                for qb in range(NQB):
                    nc.tensor.matmul(
                        psum_vT[qb][:],
                        lhsT=wv_sb[:, ec, :],
                        rhs=xT_sb[:, ec, qb * QB:(qb + 1) * QB],
                        start=(ec == 0), stop=(ec == EC - 1),
                    )
            for qb in range(NQB):
                sl = slice(qb * QB, (qb + 1) * QB)
                nc.vector.tensor_copy(qkT_sb[:, sl], psum_qk[qb][:])
                nc.sync.dma_start(kT_sb[:, sl], qkT_sb[H:P, sl])
                nc.vector.tensor_copy(vT_sb[:, sl], psum_vT[qb][:])

        # v^T -> natural v via PE transposes
        with tc.tile_pool(name="ps_vtr", bufs=2, space="PSUM") as pvtr:
            for t in range(NT):
                pt = pvtr.tile([P, H], F32)
                nc.tensor.transpose(
                    pt[:], vT_sb[:, t * P:(t + 1) * P], ident[0:H, 0:H])
                nc.vector.tensor_copy(v_sb[:, t, 0:H], pt[:])
        nc.gpsimd.memset(v_sb[:, :, H:H + 1], 1.0)

        # ---------------- Phase 2: attention ----------------
        with tc.tile_pool(name="expp", bufs=4) as ep, \
             tc.tile_pool(name="smalls", bufs=4) as sp, \
             tc.tile_pool(name="ps_s", bufs=3, space="PSUM") as ps, \
             tc.tile_pool(name="ps_o", bufs=2, space="PSUM") as po, \
             tc.tile_pool(name="ps_t", bufs=2, space="PSUM") as ptr:
            for qb in range(NQB):
                qsl = slice(qb * QB, (qb + 1) * QB)
                nkc = (qb + 1) * (QB // P)
                psum_o = po.tile([H + 1, QB], F32)
                for kc in range(nkc):
                    psum_s = ps.tile([P, QB], F32)
                    nc.tensor.matmul(
                        psum_s[:],
                        lhsT=kT_sb[:, kc * P:(kc + 1) * P],
                        rhs=qkT_sb[0:H, qsl],
                        start=True, stop=True,
                    )
                    et = ep.tile([P, QB], F32)
                    nc.scalar.activation(
                        et[:], psum_s[:],
                        mybir.ActivationFunctionType.Exp, scale=1.0 / H)
                    off = kc * P - qb * QB
                    if off >= 0:
                        # keep where q >= k: (-off) + j - p >= 0
                        nc.gpsimd.affine_select(
                            out=et[:], in_=et[:],
                            compare_op=mybir.AluOpType.is_ge,
                            fill=0.0, base=-off,
                            channel_multiplier=-1,
                            pattern=[[1, QB]],
                        )
                    nc.tensor.matmul(
                        psum_o[:],
                        lhsT=v_sb[:, kc, :],
                        rhs=et[:],
                        start=(kc == 0), stop=(kc == nkc - 1),
                    )
                oT = sp.tile([H + 1, QB], F32, tag="oT")
                nc.vector.tensor_copy(oT[:], psum_o[:])
                for j in range(QB // P):
                    pt2 = ptr.tile([P, H + 1], F32)
                    nc.tensor.transpose(
                        pt2[:], oT[:, j * P:(j + 1) * P],
                        ident[0:H + 1, 0:H + 1])
                    rec = sp.tile([P, 1], F32, tag="rec")
                    nc.vector.reciprocal(rec[:], pt2[:, H:H + 1])
                    t = qb * (QB // P) + j
                    nc.vector.tensor_scalar_mul(out_sb[:, t, :], pt2[:, 0:H], rec[:])
                nc.sync.dma_start(
                    out_d.rearrange("(t p) h -> p t h", p=P)[
                        :, qb * (QB // P):(qb + 1) * (QB // P), :],
                    out_sb[:, qb * (QB // P):(qb + 1) * (QB // P), :],
                )


def build_bass(s=S, e_dim=E, n_cores=B):
    nc = bacc.Bacc(
        "TRN2", target_bir_lowering=False, debug=False, num_devices=n_cores)
    xT_d = nc.dram_tensor("xT", [e_dim, s], F32, kind="ExternalInput").ap()
    wqk_d = nc.dram_tensor("wqk", [e_dim, 2 * H], F32, kind="ExternalInput").ap()
    wv_d = nc.dram_tensor("wv", [e_dim, H], F32, kind="ExternalInput").ap()
    out_d = nc.dram_tensor("out", [s, H], F32, kind="ExternalOutput").ap()
    with tile.TileContext(nc) as tc:
        build_kernel_body(tc, xT_d, wqk_d, wv_d, out_d, s=s, e_dim=e_dim)
    nc.compile()
    return nc


_nc_cache = None


def _ensure_ntff_hook():
    """Dev-only: provide the antenv.axon_hooks shim so trace=True can
    capture NTFF profiles through libaxon_pjrt.so in this container."""
    import sys
    import types
    import ctypes
    import contextlib

    try:
        from antenv.axon_hooks import get_axon_ntff_profile_hook  # noqa
        return
    except ImportError:
        pass
    import antenv

    mod = types.ModuleType("antenv.axon_hooks")
    _h = [None]
    mod.set_axon_ntff_profile_hook = lambda h: _h.__setitem__(0, h)
    mod.get_axon_ntff_profile_hook = lambda: _h[0]
    sys.modules["antenv.axon_hooks"] = mod
    antenv.axon_hooks = mod

    so_path = "/opt/axon/libaxon_pjrt.so"
    lib = ctypes.CDLL(so_path)
    if not hasattr(lib, "axon_start_nrt_profile"):
        return
    lib.axon_start_nrt_profile.argtypes = [
        ctypes.POINTER(ctypes.c_int64), ctypes.c_size_t]
    lib.axon_start_nrt_profile.restype = ctypes.c_int64
    lib.axon_stop_nrt_profile.argtypes = [ctypes.c_char_p]
    lib.axon_stop_nrt_profile.restype = ctypes.c_int64

    @contextlib.contextmanager
    def _hook(output_dir, device_ids):
        import jax
        jax.devices()
        if device_ids:
            ids = (ctypes.c_int64 * len(device_ids))(*device_ids)
            rc = lib.axon_start_nrt_profile(ids, len(device_ids))
        else:
            rc = lib.axon_start_nrt_profile(None, 0)
        if rc != 0:
            raise RuntimeError(f"axon_start_nrt_profile rc={rc}")
        try:
            yield
        finally:
            n = lib.axon_stop_nrt_profile(str(output_dir).encode())
            print(f"profile: {n} file(s) written to {output_dir}")

    mod.set_axon_ntff_profile_hook(_hook)

    # no bucket access in this container; keep artifacts local
    import concourse.bass_utils as bu
    bu.upload_artifacts = lambda tmpdir: tmpdir


def kernel(x, Wq, Wk, Wv):
    global _nc_cache
    x = np.asarray(x, dtype=np.float32)
    Wq = np.asarray(Wq, dtype=np.float32)
    Wk = np.asarray(Wk, dtype=np.float32)
    Wv = np.asarray(Wv, dtype=np.float32)

    if _nc_cache is None:
        _nc_cache = build_bass()
    nc = _nc_cache

    wqk = np.ascontiguousarray(np.concatenate([Wq, Wk], axis=1))
    wv = np.ascontiguousarray(Wv)
    in_maps = []
    for b in range(B):
        in_maps.append({
            "xT": np.ascontiguousarray(x[b].T),
            "wqk": wqk,
            "wv": wv,
        })

    trace = bool(int(os.environ.get("ATTN_TRACE", "0")))
    if trace:
        _ensure_ntff_hook()
    res = run_bass_kernel_spmd(
        nc, in_maps, core_ids=list(range(B)), trace=trace)
    if trace and res.exec_time_ns is not None:
        print(f"HW exec time: {res.exec_time_ns} ns")
        kernel.last_exec_time_ns = res.exec_time_ns
        kernel.last_results = res
    out = np.stack([res.results[b]["out"] for b in range(B)], axis=0)
    return out


# revision 6
# speedup vs baseline: 2.1550x; 2.1550x over previous
"""Causal single-head self-attention on 8 Trainium2 NeuronCores.

Problem: x:[8,2048,1024], Wq/Wk/Wv:[1024,64] ->
    out[b] = softmax(tril(x[b]Wq (x[b]Wk)^T / 64)) @ (x[b]Wv)   [8,2048,64]

Sharding: data-parallel over batch -- core b gets batch element b.
Weights replicated.

Per-core algorithm (layouts chosen so no on-device transposes of x or of
the attention matrix are needed; fp32 matmuls on TRN2 lower to 2x
LOW_HIGH PE passes, so all matmul operands are bf16 with fp32 PSUM
accumulation -- measured end-to-end rel err ~2.6e-3):
  - host pre-transposes x[b] -> xT [E, S] and casts to bf16
  - per q-block of 512 (pipelined with the xT DMA):
      qkT[128, qb]: rows 0:64 = q^T, 64:128 = k^T via packed projection
      (lhsT=[Wq|Wk][e]); kT DMA-shifted to partitions 0:64; v^T
      projected likewise then PE-transposed to natural v[s,64] bf16 and
      augmented with a ones column -> v_aug[s, 65]
      attention: scores^T[k-chunk, q-block] = kT_chunk.T @ qT_block
      (K=64, fp32 psum); exp via ACT (scale=1/64, fp32 in, bf16 out);
      causal handled by skipping above-diagonal chunks, truncating the
      q-range of diagonal chunks, and a gpsimd affine_select triangular
      mask; out^T psum[65, qb] += v_aug[kc].T @ expT; row 64 = softmax
      denominators (ones-column trick); PE-transpose out^T -> [128, 65],
      multiply rows 0:64 by reciprocal of col 64 (per-partition scalar);
      DMA natural [S, 64] fp32 out.
"""

import os
from contextlib import ExitStack

import numpy as np

import concourse.bass as bass
import concourse.mybir as mybir
import concourse.tile as tile
from concourse import bacc
from concourse.bass_utils import run_bass_kernel_spmd
from concourse.masks import make_identity

B, S, E, H = 8, 2048, 1024, 64
P = 128
QB = 512  # q-block (psum free dim)
F32 = mybir.dt.float32
BF16 = mybir.dt.bfloat16


def build_kernel_body(tc, xT_d, wqk_d, wv_d, out_d, s=S, e_dim=E):
    nc = tc.nc
    EC = e_dim // P  # e-chunks
    NQB = s // QB    # q-blocks
    NT = s // P      # s-tiles of 128
    KPQ = QB // P    # k-chunks per q-block (4)

    ctx = ExitStack()
    with ctx:
        const = ctx.enter_context(tc.tile_pool(name="const", bufs=1))
        big = ctx.enter_context(tc.tile_pool(name="big", bufs=1))

        wqk_sb = const.tile([P, EC, 2 * H], BF16)
        nc.sync.dma_start(wqk_sb[:], wqk_d.rearrange("(c p) m -> p c m", p=P))
        wv_sb = const.tile([P, EC, H], BF16)
        nc.sync.dma_start(wv_sb[:], wv_d.rearrange("(c p) m -> p c m", p=P))
        ident_bf = const.tile([P, P], BF16)
        make_identity(nc, ident_bf[:])
        ident32 = const.tile([H + 1, H + 1], F32)
        make_identity(nc, ident32[:])

        # xT blocks DMAed q-block-major so attention can start early
        xT_sb = big.tile([P, EC, s], BF16)
        for qb in range(NQB):
            for ec in range(EC):
                nc.sync.dma_start(
                    xT_sb[:, ec, qb * QB:(qb + 1) * QB],
                    xT_d[ec * P:(ec + 1) * P, qb * QB:(qb + 1) * QB])

        qkT_sb = big.tile([P, s], BF16)  # rows 0:64 qT, rows 64:128 kT
        kT_sb = big.tile([H, s], BF16)   # kT at base partition 0
        vT_sb = big.tile([H, s], BF16)
        v_sb = big.tile([P, NT, H + 1], BF16)  # natural v + ones col
        out_sb = big.tile([P, NT, H], F32)

        nc.gpsimd.memset(v_sb[:, :, H:H + 1], 1.0)

        pqk = ctx.enter_context(tc.tile_pool(name="ps_qk", bufs=1, space="PSUM"))
        pvt = ctx.enter_context(tc.tile_pool(name="ps_vt", bufs=1, space="PSUM"))
        ps = ctx.enter_context(tc.tile_pool(name="ps_s", bufs=2, space="PSUM"))
        po = ctx.enter_context(tc.tile_pool(name="ps_o", bufs=1, space="PSUM"))
        ptr = ctx.enter_context(tc.tile_pool(name="ps_tr", bufs=1, space="PSUM"))
        ep = ctx.enter_context(tc.tile_pool(name="expp", bufs=3))
        sp = ctx.enter_context(tc.tile_pool(name="smalls", bufs=4))

        for qb in range(NQB):
            qsl = slice(qb * QB, (qb + 1) * QB)
            # ---- projections for this q-block ----
            psum_qk = pqk.tile([P, QB], F32, tag="qk")
            psum_vT = pvt.tile([H, QB], F32, tag="vt")
            for ec in range(EC):
                nc.tensor.matmul(
                    psum_qk[:], lhsT=wqk_sb[:, ec, :],
                    rhs=xT_sb[:, ec, qsl],
                    start=(ec == 0), stop=(ec == EC - 1))
                nc.tensor.matmul(
                    psum_vT[:], lhsT=wv_sb[:, ec, :],
                    rhs=xT_sb[:, ec, qsl],
                    start=(ec == 0), stop=(ec == EC - 1))
            nc.vector.tensor_copy(qkT_sb[:, qsl], psum_qk[:])
            nc.sync.dma_start(kT_sb[:, qsl], qkT_sb[H:P, qsl])
            nc.vector.tensor_copy(vT_sb[:, qsl], psum_vT[:])
            for t in range(qb * KPQ, (qb + 1) * KPQ):
                pvtr = ptr.tile([P, H], BF16, tag="tr")
                nc.tensor.transpose(
                    pvtr[:], vT_sb[:, t * P:(t + 1) * P], ident_bf[0:H, 0:H])
                nc.vector.tensor_copy(v_sb[:, t, 0:H], pvtr[:])

            # ---- attention for this q-block ----
            nkc = (qb + 1) * KPQ
            psum_o = po.tile([H + 1, QB], F32)
            for pr in range(nkc // 2):
                kc0, kc1 = 2 * pr, 2 * pr + 1
                o0 = max(0, kc0 * P - qb * QB)
                o1 = max(0, kc1 * P - qb * QB)
                psum_pr = ps.tile([P, 2, QB], F32)
                for i, (kc, o) in enumerate(((kc0, o0), (kc1, o1))):
                    nc.tensor.matmul(
                        psum_pr[:, i, o:],
                        lhsT=kT_sb[:, kc * P:(kc + 1) * P],
                        rhs=qkT_sb[0:H, qsl][:, o:],
                        start=True, stop=True)
                et = ep.tile([P, 2, QB], BF16)
                if o0 == o1:
                    # off-diagonal pair: one big exp over both banks
                    nc.scalar.activation(
                        et[:, :, o0:], psum_pr[:, :, o0:],
                        mybir.ActivationFunctionType.Exp, scale=1.0 / H)
                else:
                    for i, (kc, o) in enumerate(((kc0, o0), (kc1, o1))):
                        nc.scalar.activation(
                            et[:, i, o:], psum_pr[:, i, o:],
                            mybir.ActivationFunctionType.Exp, scale=1.0 / H)
                for i, (kc, o) in enumerate(((kc0, o0), (kc1, o1))):
                    if kc * P - qb * QB >= 0:
                        # diagonal chunk: keep where q >= k, i.e. j - p >= 0
                        nc.gpsimd.affine_select(
                            out=et[:, i, o:], in_=et[:, i, o:],
                            compare_op=mybir.AluOpType.is_ge,
                            fill=0.0, base=0,
                            channel_multiplier=-1,
                            pattern=[[1, QB - o]])
                    nc.tensor.matmul(
                        psum_o[:, o:],
                        lhsT=v_sb[:, kc, :],
                        rhs=et[:, i, o:],
                        start=(kc == 0), stop=(kc == nkc - 1))

            # ---- normalize + store ----
            oT = sp.tile([H + 1, QB], F32, tag="oT")
            nc.vector.tensor_copy(oT[:], psum_o[:])
            for j in range(KPQ):
                pt2 = ptr.tile([P, H + 1], F32, tag="tr")
                nc.tensor.transpose(
                    pt2[:], oT[:, j * P:(j + 1) * P], ident32[:])
                rec = sp.tile([P, 1], F32, tag="rec")
                nc.vector.reciprocal(rec[:], pt2[:, H:H + 1])
                t = qb * KPQ + j
                nc.vector.tensor_scalar_mul(out_sb[:, t, :], pt2[:, 0:H], rec[:])
            nc.sync.dma_start(
                out_d.rearrange("(t p) h -> p t h", p=P)[
                    :, qb * KPQ:(qb + 1) * KPQ, :],
                out_sb[:, qb * KPQ:(qb + 1) * KPQ, :])


def build_bass(s=S, e_dim=E, n_cores=B):
    nc = bacc.Bacc(
        "TRN2", target_bir_lowering=False, debug=False, num_devices=n_cores)
    xT_d = nc.dram_tensor("xT", [e_dim, s], BF16, kind="ExternalInput").ap()
    wqk_d = nc.dram_tensor("wqk", [e_dim, 2 * H], BF16, kind="ExternalInput").ap()
    wv_d = nc.dram_tensor("wv", [e_dim, H], BF16, kind="ExternalInput").ap()
    out_d = nc.dram_tensor("out", [s, H], F32, kind="ExternalOutput").ap()
    with tile.TileContext(nc) as tc:
        build_kernel_body(tc, xT_d, wqk_d, wv_d, out_d, s=s, e_dim=e_dim)
    nc.compile()
    return nc


_nc_cache = None


def _ensure_ntff_hook():
    """Dev-only: provide the antenv.axon_hooks shim so trace=True can
    capture NTFF profiles through libaxon_pjrt.so in this container."""
    import sys
    import types
    import ctypes
    import contextlib

    try:
        from antenv.axon_hooks import get_axon_ntff_profile_hook  # noqa
        return
    except ImportError:
        pass
    import antenv

    mod = types.ModuleType("antenv.axon_hooks")
    _h = [None]
    mod.set_axon_ntff_profile_hook = lambda h: _h.__setitem__(0, h)
    mod.get_axon_ntff_profile_hook = lambda: _h[0]
    sys.modules["antenv.axon_hooks"] = mod
    antenv.axon_hooks = mod

    so_path = "/opt/axon/libaxon_pjrt.so"
    lib = ctypes.CDLL(so_path)
    if not hasattr(lib, "axon_start_nrt_profile"):
        return
    lib.axon_start_nrt_profile.argtypes = [
        ctypes.POINTER(ctypes.c_int64), ctypes.c_size_t]
    lib.axon_start_nrt_profile.restype = ctypes.c_int64
    lib.axon_stop_nrt_profile.argtypes = [ctypes.c_char_p]
    lib.axon_stop_nrt_profile.restype = ctypes.c_int64

    @contextlib.contextmanager
    def _hook(output_dir, device_ids):
        import jax
        jax.devices()
        if device_ids:
            ids = (ctypes.c_int64 * len(device_ids))(*device_ids)
            rc = lib.axon_start_nrt_profile(ids, len(device_ids))
        else:
            rc = lib.axon_start_nrt_profile(None, 0)
        if rc != 0:
            raise RuntimeError(f"axon_start_nrt_profile rc={rc}")
        try:
            yield
        finally:
            n = lib.axon_stop_nrt_profile(str(output_dir).encode())
            print(f"profile: {n} file(s) written to {output_dir}")

    mod.set_axon_ntff_profile_hook(_hook)

    # no bucket access in this container; keep artifacts local
    import concourse.bass_utils as bu
    bu.upload_artifacts = lambda tmpdir: tmpdir


def kernel(x, Wq, Wk, Wv):
    global _nc_cache
    import ml_dtypes
    bf = ml_dtypes.bfloat16

    x = np.asarray(x, dtype=np.float32)
    Wq = np.asarray(Wq, dtype=np.float32)
    Wk = np.asarray(Wk, dtype=np.float32)
    Wv = np.asarray(Wv, dtype=np.float32)

    if _nc_cache is None:
        _nc_cache = build_bass()
    nc = _nc_cache

    wqk = np.ascontiguousarray(
        np.concatenate([Wq, Wk], axis=1).astype(bf))
    wv = np.ascontiguousarray(Wv.astype(bf))
    in_maps = []
    for b in range(B):
        in_maps.append({
            "xT": np.ascontiguousarray(x[b].T.astype(bf)),
            "wqk": wqk,
            "wv": wv,
        })

    trace = bool(int(os.environ.get("ATTN_TRACE", "0")))
    if trace:
        _ensure_ntff_hook()
    res = run_bass_kernel_spmd(
        nc, in_maps, core_ids=list(range(B)), trace=trace)
    if trace and res.exec_time_ns is not None:
        print(f"HW exec time: {res.exec_time_ns} ns")
        kernel.last_exec_time_ns = res.exec_time_ns
        kernel.last_results = res
    out = np.stack([res.results[b]["out"] for b in range(B)], axis=0)
    return out


# revision 9
# speedup vs baseline: 2.2602x; 1.0488x over previous
"""Causal single-head self-attention on 8 Trainium2 NeuronCores.

Problem: x:[8,2048,1024], Wq/Wk/Wv:[1024,64] ->
    out[b] = softmax(tril(x[b]Wq (x[b]Wk)^T / 64)) @ (x[b]Wv)   [8,2048,64]

Sharding: data-parallel over batch -- core b gets batch element b.
Weights replicated.

Per-core algorithm (layouts chosen so no on-device transposes of x or of
the attention matrix are needed; fp32 matmuls on TRN2 lower to 2x
LOW_HIGH PE passes, so all matmul operands are bf16 with fp32 PSUM
accumulation -- measured end-to-end rel err ~2.6e-3):
  - host pre-transposes x[b] -> xT [E, S] and casts to bf16
  - per q-block of 512 (pipelined with the xT DMA):
      qkT[128, qb]: rows 0:64 = q^T, 64:128 = k^T via packed projection
      (lhsT=[Wq|Wk][e]); kT DMA-shifted to partitions 0:64; v^T
      projected likewise then PE-transposed to natural v[s,64] bf16 and
      augmented with a ones column -> v_aug[s, 65]
      attention: scores^T[k-chunk, q-block] = kT_chunk.T @ qT_block
      (K=64, fp32 psum); exp via ACT (scale=1/64, fp32 in, bf16 out);
      causal handled by skipping above-diagonal chunks, truncating the
      q-range of diagonal chunks, and a gpsimd affine_select triangular
      mask; out^T psum[65, qb] += v_aug[kc].T @ expT; row 64 = softmax
      denominators (ones-column trick); PE-transpose out^T -> [128, 65],
      multiply rows 0:64 by reciprocal of col 64 (per-partition scalar);
      DMA natural [S, 64] fp32 out.
"""

import os
from contextlib import ExitStack

import numpy as np

import concourse.bass as bass
import concourse.mybir as mybir
import concourse.tile as tile
from concourse import bacc
from concourse.bass_utils import run_bass_kernel_spmd
from concourse.masks import make_identity

B, S, E, H = 8, 2048, 1024, 64
P = 128
QB = 512  # q-block (psum free dim)
F32 = mybir.dt.float32
BF16 = mybir.dt.bfloat16


def build_kernel_body(tc, xT_d, wqk_d, wv_d, out_d, s=S, e_dim=E):
    nc = tc.nc
    EC = e_dim // P  # e-chunks
    NQB = s // QB    # q-blocks
    NT = s // P      # s-tiles of 128
    KPQ = QB // P    # k-chunks per q-block (4)

    ctx = ExitStack()
    with ctx:
        const = ctx.enter_context(tc.tile_pool(name="const", bufs=1))
        big = ctx.enter_context(tc.tile_pool(name="big", bufs=1))

        wqk_sb = const.tile([P, EC, 2 * H], BF16)
        nc.sync.dma_start(wqk_sb[:], wqk_d.rearrange("(c p) m -> p c m", p=P))
        wv_sb = const.tile([P, EC, H], BF16)
        nc.sync.dma_start(wv_sb[:], wv_d.rearrange("(c p) m -> p c m", p=P))
        ident_bf = const.tile([P, P], BF16)
        make_identity(nc, ident_bf[:])
        ident32 = const.tile([H + 1, H + 1], F32)
        make_identity(nc, ident32[:])

        # xT blocks DMAed q-block-major (1 MB each) so attention starts early
        xT_sb = big.tile([P, EC, s], BF16)
        xT_r = xT_d.rearrange("(c p) s -> p c s", p=P)
        for qb in range(NQB):
            nc.sync.dma_start(
                xT_sb[:, :, qb * QB:(qb + 1) * QB],
                xT_r[:, :, qb * QB:(qb + 1) * QB])

        qkT_sb = big.tile([P, s], BF16)  # rows 0:64 qT, rows 64:128 kT
        kT_sb = big.tile([H, s], BF16)   # kT at base partition 0
        qT2_sb = big.tile([P, s], BF16)  # qT duplicated at rows 64:128
        vT_sb = big.tile([H, s], BF16)
        v_sb = big.tile([P, NT, H + 1], BF16)  # natural v + ones col
        out_sb = big.tile([P, NT, H], F32)

        nc.gpsimd.memset(v_sb[:, :, H:H + 1], 1.0)

        pqk = ctx.enter_context(tc.tile_pool(name="ps_qk", bufs=1, space="PSUM"))
        pvt = ctx.enter_context(tc.tile_pool(name="ps_vt", bufs=1, space="PSUM"))
        ps = ctx.enter_context(tc.tile_pool(name="ps_s", bufs=2, space="PSUM"))
        po = ctx.enter_context(tc.tile_pool(name="ps_o", bufs=1, space="PSUM"))
        ptr = ctx.enter_context(tc.tile_pool(name="ps_tr", bufs=1, space="PSUM"))
        ep = ctx.enter_context(tc.tile_pool(name="expp", bufs=3))
        sp = ctx.enter_context(tc.tile_pool(name="smalls", bufs=4))

        for qb in range(NQB):
            qsl = slice(qb * QB, (qb + 1) * QB)
            # ---- projections for this q-block ----
            psum_qk = pqk.tile([P, QB], F32, tag="qk")
            psum_vT = pvt.tile([H, QB], F32, tag="vt")
            for ec in range(EC):
                nc.tensor.matmul(
                    psum_qk[:], lhsT=wqk_sb[:, ec, :],
                    rhs=xT_sb[:, ec, qsl],
                    start=(ec == 0), stop=(ec == EC - 1))
                nc.tensor.matmul(
                    psum_vT[:], lhsT=wv_sb[:, ec, :],
                    rhs=xT_sb[:, ec, qsl],
                    start=(ec == 0), stop=(ec == EC - 1))
            nc.vector.tensor_copy(qkT_sb[:, qsl], psum_qk[:])
            nc.gpsimd.dma_start(kT_sb[:, qsl], qkT_sb[H:P, qsl])
            nc.gpsimd.dma_start(qT2_sb[H:P, qsl], qkT_sb[0:H, qsl])
            nc.vector.tensor_copy(vT_sb[:, qsl], psum_vT[:])
            for t in range(qb * KPQ, (qb + 1) * KPQ):
                pvtr = ptr.tile([P, H], BF16, tag="tr")
                nc.tensor.transpose(
                    pvtr[:], vT_sb[:, t * P:(t + 1) * P], ident_bf[0:H, 0:H])
                nc.vector.tensor_copy(v_sb[:, t, 0:H], pvtr[:])

            # ---- attention for this q-block ----
            nkc = (qb + 1) * KPQ
            psum_o = po.tile([H + 1, QB], F32)
            for pr in range(nkc // 2):
                kc0, kc1 = 2 * pr, 2 * pr + 1
                o0 = max(0, kc0 * P - qb * QB)
                o1 = max(0, kc1 * P - qb * QB)
                psum_pr = ps.tile([P, 2, QB], F32)
                # row-packed pair: kc0 on PE rows 0:64, kc1 on rows 64:128
                # (kT lives at rows 64:128 of qkT_sb; qT duplicated there)
                nc.tensor.matmul(
                    psum_pr[:, 0, o0:],
                    lhsT=kT_sb[:, kc0 * P:(kc0 + 1) * P],
                    rhs=qkT_sb[0:H, qsl][:, o0:],
                    start=True, stop=True)
                nc.tensor.matmul(
                    psum_pr[:, 1, o1:],
                    lhsT=qkT_sb[H:P, kc1 * P:(kc1 + 1) * P],
                    rhs=qT2_sb[H:P, qsl][:, o1:],
                    start=True, stop=True)
                et = ep.tile([P, 2, QB], BF16)
                if o0 == o1:
                    # off-diagonal pair: one big exp over both banks
                    nc.scalar.activation(
                        et[:, :, o0:], psum_pr[:, :, o0:],
                        mybir.ActivationFunctionType.Exp, scale=1.0 / H)
                else:
                    for i, (kc, o) in enumerate(((kc0, o0), (kc1, o1))):
                        nc.scalar.activation(
                            et[:, i, o:], psum_pr[:, i, o:],
                            mybir.ActivationFunctionType.Exp, scale=1.0 / H)
                for i, (kc, o) in enumerate(((kc0, o0), (kc1, o1))):
                    if kc * P - qb * QB >= 0:
                        # diagonal chunk: keep where q >= k, i.e. j - p >= 0
                        nc.gpsimd.affine_select(
                            out=et[:, i, o:], in_=et[:, i, o:],
                            compare_op=mybir.AluOpType.is_ge,
                            fill=0.0, base=0,
                            channel_multiplier=-1,
                            pattern=[[1, QB - o]])
                    nc.tensor.matmul(
                        psum_o[:, o:],
                        lhsT=v_sb[:, kc, :],
                        rhs=et[:, i, o:],
                        start=(kc == 0), stop=(kc == nkc - 1))

            # ---- normalize + store ----
            oT = sp.tile([H + 1, QB], F32, tag="oT")
            nc.vector.tensor_copy(oT[:], psum_o[:])
            for j in range(KPQ):
                pt2 = ptr.tile([P, H + 1], F32, tag="tr")
                nc.tensor.transpose(
                    pt2[:], oT[:, j * P:(j + 1) * P], ident32[:])
                rec = sp.tile([P, 1], F32, tag="rec")
                nc.vector.reciprocal(rec[:], pt2[:, H:H + 1])
                t = qb * KPQ + j
                nc.vector.tensor_scalar_mul(out_sb[:, t, :], pt2[:, 0:H], rec[:])
            nc.sync.dma_start(
                out_d.rearrange("(t p) h -> p t h", p=P)[
                    :, qb * KPQ:(qb + 1) * KPQ, :],
                out_sb[:, qb * KPQ:(qb + 1) * KPQ, :])


def build_bass(s=S, e_dim=E, n_cores=B):
    nc = bacc.Bacc(
        "TRN2", target_bir_lowering=False, debug=False, num_devices=n_cores)
    xT_d = nc.dram_tensor("xT", [e_dim, s], BF16, kind="ExternalInput").ap()
    wqk_d = nc.dram_tensor("wqk", [e_dim, 2 * H], BF16, kind="ExternalInput").ap()
    wv_d = nc.dram_tensor("wv", [e_dim, H], BF16, kind="ExternalInput").ap()
    out_d = nc.dram_tensor("out", [s, H], F32, kind="ExternalOutput").ap()
    with tile.TileContext(nc) as tc:
        build_kernel_body(tc, xT_d, wqk_d, wv_d, out_d, s=s, e_dim=e_dim)
    nc.compile()
    return nc


_nc_cache = None


def _ensure_ntff_hook():
    """Dev-only: provide the antenv.axon_hooks shim so trace=True can
    capture NTFF profiles through libaxon_pjrt.so in this container."""
    import sys
    import types
    import ctypes
    import contextlib

    try:
        from antenv.axon_hooks import get_axon_ntff_profile_hook  # noqa
        return
    except ImportError:
        pass
    import antenv

    mod = types.ModuleType("antenv.axon_hooks")
    _h = [None]
    mod.set_axon_ntff_profile_hook = lambda h: _h.__setitem__(0, h)
    mod.get_axon_ntff_profile_hook = lambda: _h[0]
    sys.modules["antenv.axon_hooks"] = mod
    antenv.axon_hooks = mod

    so_path = "/opt/axon/libaxon_pjrt.so"
    lib = ctypes.CDLL(so_path)
    if not hasattr(lib, "axon_start_nrt_profile"):
        return
    lib.axon_start_nrt_profile.argtypes = [
        ctypes.POINTER(ctypes.c_int64), ctypes.c_size_t]
    lib.axon_start_nrt_profile.restype = ctypes.c_int64
    lib.axon_stop_nrt_profile.argtypes = [ctypes.c_char_p]
    lib.axon_stop_nrt_profile.restype = ctypes.c_int64

    @contextlib.contextmanager
    def _hook(output_dir, device_ids):
        import jax
        jax.devices()
        if device_ids:
            ids = (ctypes.c_int64 * len(device_ids))(*device_ids)
            rc = lib.axon_start_nrt_profile(ids, len(device_ids))
        else:
            rc = lib.axon_start_nrt_profile(None, 0)
        if rc != 0:
            raise RuntimeError(f"axon_start_nrt_profile rc={rc}")
        try:
            yield
        finally:
            n = lib.axon_stop_nrt_profile(str(output_dir).encode())
            print(f"profile: {n} file(s) written to {output_dir}")

    mod.set_axon_ntff_profile_hook(_hook)

    # no bucket access in this container; keep artifacts local
    import concourse.bass_utils as bu
    bu.upload_artifacts = lambda tmpdir: tmpdir


def kernel(x, Wq, Wk, Wv):
    global _nc_cache
    import ml_dtypes
    bf = ml_dtypes.bfloat16

    x = np.asarray(x, dtype=np.float32)
    Wq = np.asarray(Wq, dtype=np.float32)
    Wk = np.asarray(Wk, dtype=np.float32)
    Wv = np.asarray(Wv, dtype=np.float32)

    if _nc_cache is None:
        _nc_cache = build_bass()
    nc = _nc_cache

    wqk = np.ascontiguousarray(
        np.concatenate([Wq, Wk], axis=1).astype(bf))
    wv = np.ascontiguousarray(Wv.astype(bf))
    in_maps = []
    for b in range(B):
        in_maps.append({
            "xT": np.ascontiguousarray(x[b].T.astype(bf)),
            "wqk": wqk,
            "wv": wv,
        })

    trace = bool(int(os.environ.get("ATTN_TRACE", "0")))
    if trace:
        _ensure_ntff_hook()
    res = run_bass_kernel_spmd(
        nc, in_maps, core_ids=list(range(B)), trace=trace)
    if trace and res.exec_time_ns is not None:
        print(f"HW exec time: {res.exec_time_ns} ns")
        kernel.last_exec_time_ns = res.exec_time_ns
        kernel.last_results = res
    out = np.stack([res.results[b]["out"] for b in range(B)], axis=0)
    return out


# revision 13
# speedup vs baseline: 2.3601x; 1.0442x over previous
"""Causal single-head self-attention on 8 Trainium2 NeuronCores.

Problem: x:[8,2048,1024], Wq/Wk/Wv:[1024,64] ->
    out[b] = softmax(tril(x[b]Wq (x[b]Wk)^T / 64)) @ (x[b]Wv)   [8,2048,64]

Sharding: data-parallel over batch -- core b gets batch element b.
Weights replicated.

Per-core algorithm (layouts chosen so no on-device transposes of x or of
the attention matrix are needed; fp32 matmuls on TRN2 lower to 2x
LOW_HIGH PE passes, so all matmul operands are bf16 with fp32 PSUM
accumulation -- measured end-to-end rel err ~2.6e-3):
  - host pre-transposes x[b] -> xT [E, S] and casts to bf16
  - per q-block of 512 (pipelined with the xT DMA):
      qkT[128, qb]: rows 0:64 = q^T, 64:128 = k^T via packed projection
      (lhsT=[Wq|Wk][e]); kT DMA-shifted to partitions 0:64; v^T
      projected likewise then PE-transposed to natural v[s,64] bf16 and
      augmented with a ones column -> v_aug[s, 65]
      attention: scores^T[k-chunk, q-block] = kT_chunk.T @ qT_block
      (K=64, fp32 psum); exp via ACT (scale=1/64, fp32 in, bf16 out);
      causal handled by skipping above-diagonal chunks, truncating the
      q-range of diagonal chunks, and a gpsimd affine_select triangular
      mask; out^T psum[65, qb] += v_aug[kc].T @ expT; row 64 = softmax
      denominators (ones-column trick); PE-transpose out^T -> [128, 65],
      multiply rows 0:64 by reciprocal of col 64 (per-partition scalar);
      DMA natural [S, 64] fp32 out.
"""

import os
from contextlib import ExitStack

import numpy as np

import concourse.bass as bass
import concourse.mybir as mybir
import concourse.tile as tile
from concourse import bacc
from concourse.bass_utils import run_bass_kernel_spmd
from concourse.masks import make_identity

B, S, E, H = 8, 2048, 1024, 64
P = 128
QB = 512  # q-block (psum free dim)
F32 = mybir.dt.float32
BF16 = mybir.dt.bfloat16


def build_kernel_body(tc, xT_d, wqk_d, wv_d, out_d, s=S, e_dim=E):
    nc = tc.nc
    EC = e_dim // P  # e-chunks
    NQB = s // QB    # q-blocks
    NT = s // P      # s-tiles of 128
    KPQ = QB // P    # k-chunks per q-block (4)

    ctx = ExitStack()
    with ctx:
        const = ctx.enter_context(tc.tile_pool(name="const", bufs=1))
        big = ctx.enter_context(tc.tile_pool(name="big", bufs=1))

        wqk_sb = const.tile([P, EC, 2 * H], BF16)
        nc.sync.dma_start(wqk_sb[:], wqk_d.rearrange("(c p) m -> p c m", p=P))
        wv_sb = const.tile([P, EC, H], BF16)
        nc.sync.dma_start(wv_sb[:], wv_d.rearrange("(c p) m -> p c m", p=P))
        ident_bf = const.tile([P, P], BF16)
        make_identity(nc, ident_bf[:])
        ident32 = const.tile([H + 1, H + 1], F32)
        make_identity(nc, ident32[:])

        # xT blocks DMAed q-block-major so attention starts early; the
        # first q-block is split per e-chunk so the very first projection
        # matmul can start as soon as 128 KB have landed
        xT_sb = big.tile([P, EC, s], BF16)
        xT_r = xT_d.rearrange("(c p) s -> p c s", p=P)
        for ec in range(EC):
            nc.sync.dma_start(
                xT_sb[:, ec, 0:QB], xT_r[:, ec, 0:QB])
        for qb in range(1, NQB):
            nc.sync.dma_start(
                xT_sb[:, :, qb * QB:(qb + 1) * QB],
                xT_r[:, :, qb * QB:(qb + 1) * QB])

        qkT_sb = big.tile([P, s], BF16)  # rows 0:64 qT, rows 64:128 kT
        kT_sb = big.tile([H, s], BF16)   # kT at base partition 0
        qT2_sb = big.tile([P, s], BF16)  # qT duplicated at rows 64:128
        vT_sb = big.tile([H, s], BF16)
        v_sb = big.tile([P, NT, H + 1], BF16)  # natural v + ones col
        out_sb = big.tile([P, NT, H], F32)

        nc.gpsimd.memset(v_sb[:, :, H:H + 1], 1.0)

        # PSUM budget (8 banks): pqk 1 + pvt 1 + ps 3 + po 1 + ptr 2 = 8
        pqk = ctx.enter_context(tc.tile_pool(name="ps_qk", bufs=1, space="PSUM"))
        pvt = ctx.enter_context(tc.tile_pool(name="ps_vt", bufs=1, space="PSUM"))
        ps = ctx.enter_context(tc.tile_pool(name="ps_s", bufs=3, space="PSUM"))
        po = ctx.enter_context(tc.tile_pool(name="ps_o", bufs=1, space="PSUM"))
        ptr = ctx.enter_context(tc.tile_pool(name="ps_tr", bufs=2, space="PSUM"))
        ep = ctx.enter_context(tc.tile_pool(name="expp", bufs=6))
        sp = ctx.enter_context(tc.tile_pool(name="smalls", bufs=4))

        psum_o_pend = [None] * NQB  # psum_o awaiting normalize

        def normalize(qb):
            # out^T[65, QB] -> PE-transpose to [128, 65]; col 64 holds the
            # softmax denominators; divide and store natural [S, 64]
            psum_o = psum_o_pend[qb]
            oT = sp.tile([H + 1, QB], F32, tag="oT")
            nc.vector.tensor_copy(oT[:], psum_o[:])
            for j in range(KPQ):
                pt2 = ptr.tile([P, H + 1], F32, tag="tr")
                nc.tensor.transpose(
                    pt2[:], oT[:, j * P:(j + 1) * P], ident32[:])
                rec = sp.tile([P, 1], F32, tag="rec")
                nc.vector.reciprocal(rec[:], pt2[:, H:H + 1])
                t = qb * KPQ + j
                nc.vector.tensor_scalar_mul(
                    out_sb[:, t, :], pt2[:, 0:H], rec[:])
            nc.sync.dma_start(
                out_d.rearrange("(t p) h -> p t h", p=P)[
                    :, qb * KPQ:(qb + 1) * KPQ, :],
                out_sb[:, qb * KPQ:(qb + 1) * KPQ, :])

        for qb in range(NQB):
            qsl = slice(qb * QB, (qb + 1) * QB)
            # ---- projections for this q-block ----
            psum_qk = pqk.tile([P, QB], F32, tag="qk")
            psum_vT = pvt.tile([H, QB], F32, tag="vt")
            for ec in range(EC):
                nc.tensor.matmul(
                    psum_qk[:], lhsT=wqk_sb[:, ec, :],
                    rhs=xT_sb[:, ec, qsl],
                    start=(ec == 0), stop=(ec == EC - 1))
                nc.tensor.matmul(
                    psum_vT[:], lhsT=wv_sb[:, ec, :],
                    rhs=xT_sb[:, ec, qsl],
                    start=(ec == 0), stop=(ec == EC - 1))
            nc.vector.tensor_copy(qkT_sb[:, qsl], psum_qk[:])
            nc.gpsimd.dma_start(kT_sb[:, qsl], qkT_sb[H:P, qsl])
            nc.gpsimd.dma_start(qT2_sb[H:P, qsl], qkT_sb[0:H, qsl])
            nc.vector.tensor_copy(vT_sb[:, qsl], psum_vT[:])
            for t in range(qb * KPQ, (qb + 1) * KPQ):
                pvtr = ptr.tile([P, H], BF16, tag="tr")
                nc.tensor.transpose(
                    pvtr[:], vT_sb[:, t * P:(t + 1) * P], ident_bf[0:H, 0:H])
                nc.vector.tensor_copy(v_sb[:, t, 0:H], pvtr[:])

            # previous q-block's normalize overlaps this one's attention
            if qb > 0:
                normalize(qb - 1)

            # ---- attention for this q-block ----
            nkc = (qb + 1) * KPQ
            psum_o = po.tile([H + 1, QB], F32)
            psum_o_pend[qb] = psum_o
            for pr in range(nkc // 2):
                kc0, kc1 = 2 * pr, 2 * pr + 1
                # row-packed pair: kc0 on PE rows 0:64, kc1 on rows 64:128
                # (kT lives at rows 64:128 of qkT_sb; qT duplicated there);
                # the two matmuls run concurrently on different row groups
                psum_prs = []
                offs = []
                for i, kc in enumerate((kc0, kc1)):
                    o = max(0, kc * P - qb * QB)
                    offs.append(o)
                    psum_s = ps.tile([P, QB], F32, tag="sc")
                    psum_prs.append(psum_s)
                    if i == 0:
                        nc.tensor.matmul(
                            psum_s[:, o:],
                            lhsT=kT_sb[:, kc * P:(kc + 1) * P],
                            rhs=qkT_sb[0:H, qsl][:, o:],
                            start=True, stop=True)
                    else:
                        nc.tensor.matmul(
                            psum_s[:, o:],
                            lhsT=qkT_sb[H:P, kc * P:(kc + 1) * P],
                            rhs=qT2_sb[H:P, qsl][:, o:],
                            start=True, stop=True)
                for i, (kc, o) in enumerate(((kc0, offs[0]), (kc1, offs[1]))):
                    et = ep.tile([P, QB], BF16)
                    nc.scalar.activation(
                        et[:, o:], psum_prs[i][:, o:],
                        mybir.ActivationFunctionType.Exp, scale=1.0 / H)
                    if kc * P - qb * QB >= 0:
                        # diagonal chunk: keep where q >= k, i.e. j - p >= 0
                        nc.gpsimd.affine_select(
                            out=et[:, o:], in_=et[:, o:],
                            compare_op=mybir.AluOpType.is_ge,
                            fill=0.0, base=0,
                            channel_multiplier=-1,
                            pattern=[[1, QB - o]])
                    nc.tensor.matmul(
                        psum_o[:, o:],
                        lhsT=v_sb[:, kc, :],
                        rhs=et[:, o:],
                        start=(kc == 0), stop=(kc == nkc - 1))

        normalize(NQB - 1)


def build_bass(s=S, e_dim=E, n_cores=B):
    nc = bacc.Bacc(
        "TRN2", target_bir_lowering=False, debug=False, num_devices=n_cores)
    xT_d = nc.dram_tensor("xT", [e_dim, s], BF16, kind="ExternalInput").ap()
    wqk_d = nc.dram_tensor("wqk", [e_dim, 2 * H], BF16, kind="ExternalInput").ap()
    wv_d = nc.dram_tensor("wv", [e_dim, H], BF16, kind="ExternalInput").ap()
    out_d = nc.dram_tensor("out", [s, H], F32, kind="ExternalOutput").ap()
    with tile.TileContext(nc) as tc:
        build_kernel_body(tc, xT_d, wqk_d, wv_d, out_d, s=s, e_dim=e_dim)
    nc.compile()
    return nc


_nc_cache = None


def _ensure_ntff_hook():
    """Dev-only: provide the antenv.axon_hooks shim so trace=True can
    capture NTFF profiles through libaxon_pjrt.so in this container."""
    import sys
    import types
    import ctypes
    import contextlib

    try:
        from antenv.axon_hooks import get_axon_ntff_profile_hook  # noqa
        return
    except ImportError:
        pass
    import antenv

    mod = types.ModuleType("antenv.axon_hooks")
    _h = [None]
    mod.set_axon_ntff_profile_hook = lambda h: _h.__setitem__(0, h)
    mod.get_axon_ntff_profile_hook = lambda: _h[0]
    sys.modules["antenv.axon_hooks"] = mod
    antenv.axon_hooks = mod

    so_path = "/opt/axon/libaxon_pjrt.so"
    lib = ctypes.CDLL(so_path)
    if not hasattr(lib, "axon_start_nrt_profile"):
        return
    lib.axon_start_nrt_profile.argtypes = [
        ctypes.POINTER(ctypes.c_int64), ctypes.c_size_t]
    lib.axon_start_nrt_profile.restype = ctypes.c_int64
    lib.axon_stop_nrt_profile.argtypes = [ctypes.c_char_p]
    lib.axon_stop_nrt_profile.restype = ctypes.c_int64

    @contextlib.contextmanager
    def _hook(output_dir, device_ids):
        import jax
        jax.devices()
        if device_ids:
            ids = (ctypes.c_int64 * len(device_ids))(*device_ids)
            rc = lib.axon_start_nrt_profile(ids, len(device_ids))
        else:
            rc = lib.axon_start_nrt_profile(None, 0)
        if rc != 0:
            raise RuntimeError(f"axon_start_nrt_profile rc={rc}")
        try:
            yield
        finally:
            n = lib.axon_stop_nrt_profile(str(output_dir).encode())
            print(f"profile: {n} file(s) written to {output_dir}")

    mod.set_axon_ntff_profile_hook(_hook)

    # no bucket access in this container; keep artifacts local
    import concourse.bass_utils as bu
    bu.upload_artifacts = lambda tmpdir: tmpdir


def kernel(x, Wq, Wk, Wv):
    global _nc_cache
    import ml_dtypes
    bf = ml_dtypes.bfloat16

    x = np.asarray(x, dtype=np.float32)
    Wq = np.asarray(Wq, dtype=np.float32)
    Wk = np.asarray(Wk, dtype=np.float32)
    Wv = np.asarray(Wv, dtype=np.float32)

    if _nc_cache is None:
        _nc_cache = build_bass()
    nc = _nc_cache

    wqk = np.ascontiguousarray(
        np.concatenate([Wq, Wk], axis=1).astype(bf))
    wv = np.ascontiguousarray(Wv.astype(bf))
    in_maps = []
    for b in range(B):
        in_maps.append({
            "xT": np.ascontiguousarray(x[b].T.astype(bf)),
            "wqk": wqk,
            "wv": wv,
        })

    trace = bool(int(os.environ.get("ATTN_TRACE", "0")))
    if trace:
        _ensure_ntff_hook()
    res = run_bass_kernel_spmd(
        nc, in_maps, core_ids=list(range(B)), trace=trace)
    if trace and res.exec_time_ns is not None:
        print(f"HW exec time: {res.exec_time_ns} ns")
        kernel.last_exec_time_ns = res.exec_time_ns
        kernel.last_results = res
    out = np.stack([res.results[b]["out"] for b in range(B)], axis=0)
    return out
